# revision 11
# baseline (speedup 1.0000x reference)
"""Bilinear decoder kernel for Trainium2 (8 NeuronCores).

score_e = sigmoid(z[row_e] @ W @ z[col_e])  for 200k edges, d=512.

v2 strategy (host->device transfer over axon is ~40MB/s, so uploads are
sharded and the tables are rebuilt on-device with AllGathers):
  - Upload per core: z shard [1280,512] bf16 (1/8 of nodes), W shard
    [64,512] bf16 (1/8 of rows), edge indices [16, 2*1568] int16.
    ~1.6MB/core vs ~41MB/core for the replicated-f32 baseline.
  - Device: AllGather W (tiny) -> full W in SBUF. Load z^T via
    dma_start_transpose, matmul ZW_c = z_c @ W for the local 1280-node
    shard (tensor engine, bf16). AllGather z -> full Z table [10240,512]
    in DRAM; AllGather ZW_c -> full ZW table.
  - Gather ZW[row_e] and Z[col_e] rows via dma_gather, per-edge dot via
    DVE mul + ACT copy-with-accumulate, sigmoid on ACT.
  - Edges sharded 25000/core; node ids are remapped on host to the
    padded AllGather layout (node n -> (n//1250)*1280 + n%1250).

Host-side work is layout-only: bf16 casts, shard slicing, index
wrap/remap, output unshard.
"""

import sys

if "/opt/trn_rl_repo" not in sys.path:
    sys.path.insert(0, "/opt/trn_rl_repo")

from dataclasses import dataclass

import numpy as np


@dataclass(frozen=True)
class Cfg:
    n_cores: int = 8
    d: int = 512              # embedding dim
    n_nodes: int = 10000      # node table rows
    e_total: int = 200000     # total edges
    gchunk: int = 512         # edges per dma_gather (SDMA packet limit:
    #                           512 rows = 32 descriptors/engine works,
    #                           1024+ faults the exec unit)

    @property
    def kb(self):
        return self.d // 128  # 4

    @property
    def nsh(self):
        return self.n_nodes // self.n_cores  # 1250 nodes per core

    @property
    def nshp(self):
        return ((self.nsh + 127) // 128) * 128  # 1280 padded

    @property
    def nblocks(self):
        return self.nshp // 128  # 10

    @property
    def ntab(self):
        return self.nshp * self.n_cores  # 10240 table rows

    @property
    def wsh(self):
        return self.d // self.n_cores  # 64 W rows per core

    @property
    def e_core(self):
        return self.e_total // self.n_cores  # 25000

    @property
    def ep_core(self):
        return ((self.e_core + 127) // 128) * 128  # 25088

    @property
    def eblocks(self):
        return self.ep_core // 128  # 196

    @property
    def idx_cols(self):
        return self.ep_core // 16  # 1568

    @property
    def chunks(self):
        out = []
        left = self.ep_core
        while left > 0:
            c = min(self.gchunk, left)
            out.append(c)
            left -= c
        return out


CFG = Cfg()


def build_kernel(cfg: Cfg):
    """Build + compile the Bacc module. Returns nc."""
    import concourse.bacc as bacc
    import concourse.mybir as mybir
    from concourse import tile

    f32 = mybir.dt.float32
    bf16 = mybir.dt.bfloat16
    i16 = mybir.dt.int16

    D, KB, NSHP, NB = cfg.d, cfg.kb, cfg.nshp, cfg.nblocks
    NTAB, WSH, IC = cfg.ntab, cfg.wsh, cfg.idx_cols
    group = [list(range(cfg.n_cores))]

    nc = bacc.Bacc(
        "TRN2", target_bir_lowering=False, debug=False, num_devices=cfg.n_cores
    )

    zin = nc.dram_tensor("zin", [NSHP, D], bf16, kind="ExternalInput")
    win = nc.dram_tensor("win", [WSH, D], bf16, kind="ExternalInput")
    eidx = nc.dram_tensor("eidx", [16, 2 * IC], i16, kind="ExternalInput")
    scores = nc.dram_tensor("scores", [128, cfg.eblocks], f32, kind="ExternalOutput")
    # Shared-scratchpad AllGather outputs: one copy in chip HBM instead of
    # eight core-local replicas (supported for AllGather with 8 cores).
    wag_out = nc.dram_tensor("wag_out", [D, D], bf16, addr_space="Shared")
    zag_out = nc.dram_tensor("zag_out", [2 * NTAB, D], bf16, addr_space="Shared")

    with tile.TileContext(nc) as tc:
        with (
            tc.tile_pool(name="const", bufs=1) as constp,
            tc.tile_pool(name="dram", bufs=1, space="DRAM") as dramp,
            tc.tile_pool(name="zwsb", bufs=2) as zwp,
            tc.tile_pool(name="rows", bufs=2) as rowsp,
            tc.tile_pool(name="cols", bufs=2) as colsp,
            tc.tile_pool(name="prod", bufs=4) as prodp,
            tc.tile_pool(name="ps", bufs=2, space="PSUM") as psp,
        ):
            # ---- DRAM bounce buffers (collectives can't touch I/O tensors) ----
            # Combined z+zw AllGather: each core contributes [z_c; zw_c]
            # [2*NSHP, D]; output is the interleaved table
            # [z_0; zw_0; z_1; zw_1; ...] that host-side index remap targets.
            wag_in = dramp.tile([WSH, D], bf16, tag="wag_in")
            zag_in = dramp.tile([2 * NSHP, D], bf16, tag="zag_in")

            nc.sync.dma_start(wag_in[:], win.ap())
            nc.sync.dma_start(zag_in[:NSHP, :], zin.ap())

            # ---- collectives (gpsimd, straight-line order) ----
            nc.gpsimd.collective_compute(
                "AllGather",
                mybir.AluOpType.bypass,
                replica_groups=group,
                ins=[wag_in.opt()],
                outs=[wag_out.ap()],
            )

            # ---- SBUF constants ----
            w_sb = constp.tile([128, KB, D], bf16, tag="w")
            nc.sync.dma_start(
                w_sb[:], wag_out.ap().rearrange("(kb p) f -> p kb f", p=128)
            )
            # z^T for the matmul's stationary operand: [128, kb, NSHP]
            zt_sb = constp.tile([128, KB, NSHP], bf16, tag="zt")
            nc.sync.dma_start_transpose(zt_sb[:], zin.ap())
            # edge indices: upload 16-partition wrap, replicate to 128
            idx_sb = constp.tile([128, 2 * IC], i16, tag="idx")
            nc.sync.dma_start(idx_sb[0:16, :], eidx.ap())
            for r in range(1, 8):
                nc.sync.dma_start(idx_sb[16 * r : 16 * (r + 1), :], idx_sb[0:16, :])
            scores_sb = constp.tile([128, cfg.eblocks], f32, tag="scores")
            sig_sb = constp.tile([128, cfg.eblocks], f32, tag="sig")
            scratch = constp.tile([128, D], f32, tag="scratch")

            # ---- phase 1: ZW_c = z_c @ W for the local node shard ----
            for nb in range(NB):
                ps = psp.tile([128, D], f32, tag="ps")
                for kb in range(KB):
                    nc.tensor.matmul(
                        ps[:],
                        lhsT=zt_sb[:, kb, nb * 128 : (nb + 1) * 128],
                        rhs=w_sb[:, kb, :],
                        start=(kb == 0),
                        stop=(kb == KB - 1),
                    )
                zw_t = zwp.tile([128, D], bf16, tag="zwt")
                nc.vector.tensor_copy(zw_t[:], ps[:])
                nc.sync.dma_start(
                    zag_in[NSHP + nb * 128 : NSHP + (nb + 1) * 128, :], zw_t[:]
                )

            nc.gpsimd.collective_compute(
                "AllGather",
                mybir.AluOpType.bypass,
                replica_groups=group,
                ins=[zag_in.opt()],
                outs=[zag_out.ap()],
            )

            # ---- phase 2: gathers + per-edge dots ----
            blk = 0
            off = 0
            for G in cfg.chunks:
                gb = G // 128
                ctile = colsp.tile([128, cfg.gchunk // 128, D], bf16, tag="ct")
                nc.gpsimd.dma_gather(
                    ctile[:, :gb, :],
                    zag_out.ap(),
                    idx_sb[:, IC + off : IC + off + G // 16],
                    num_idxs=G,
                    num_idxs_reg=G,
                    elem_size=D,
                )
                rtile = rowsp.tile([128, cfg.gchunk // 128, D], bf16, tag="rt")
                nc.gpsimd.dma_gather(
                    rtile[:, :gb, :],
                    zag_out.ap(),
                    idx_sb[:, off : off + G // 16],
                    num_idxs=G,
                    num_idxs_reg=G,
                    elem_size=D,
                )
                for b in range(gb):
                    # DVE multiply, then ACT copy-with-accumulate = free-dim sum.
                    prod = prodp.tile([128, D], f32, tag="prod")
                    nc.vector.tensor_mul(prod[:], rtile[:, b, :], ctile[:, b, :])
                    nc.scalar.activation(
                        scratch[:],
                        prod[:],
                        mybir.ActivationFunctionType.Copy,
                        accum_out=scores_sb[:, blk : blk + 1],
                    )
                    blk += 1
                off += G // 16

            # ---- sigmoid + writeback ----
            nc.scalar.activation(
                sig_sb[:], scores_sb[:], mybir.ActivationFunctionType.Sigmoid
            )
            nc.sync.dma_start(scores.ap(), sig_sb[:])

    nc.compile()
    return nc


def _wrap_idx(ids: np.ndarray, cfg: Cfg) -> np.ndarray:
    """int table-row ids [ep_core] -> [16, ep_core//16] int16 in the
    16-partition wrapped layout dma_gather expects."""
    out = np.empty((16, cfg.ep_core // 16), dtype=np.int16)
    off = 0
    for G in cfg.chunks:
        c = ids[off : off + G].reshape(G // 16, 16).T  # [16, G/16]
        out[:, off // 16 : (off + G) // 16] = c
        off += G
    return out


def prep_inputs(z_drug, weight, batch_edges, cfg: Cfg):
    """Host-side layout prep. Returns per-core input maps."""
    import ml_dtypes

    bf16 = ml_dtypes.bfloat16

    z = np.asarray(z_drug, dtype=np.float32)
    w = np.asarray(weight, dtype=np.float32)
    be = np.asarray(batch_edges)

    per_core = []
    for c in range(cfg.n_cores):
        # z shard: nodes [c*1250, (c+1)*1250), padded to 1280 rows
        zsh = np.zeros((cfg.nshp, cfg.d), dtype=bf16)
        zsh[: cfg.nsh] = z[c * cfg.nsh : (c + 1) * cfg.nsh].astype(bf16)
        # W shard: rows [c*64, (c+1)*64)
        wsh = np.ascontiguousarray(
            w[c * cfg.wsh : (c + 1) * cfg.wsh].astype(bf16)
        )
        # edge shard + remap node ids to the padded AllGather table layout
        sl = slice(c * cfg.e_core, (c + 1) * cfg.e_core)
        rids = np.zeros(cfg.ep_core, dtype=np.int64)
        cids = np.zeros(cfg.ep_core, dtype=np.int64)
        rids[: cfg.e_core] = be[0, sl]
        cids[: cfg.e_core] = be[1, sl]
        # combined table layout: [z_0; zw_0; z_1; zw_1; ...], stride 2*nshp
        rids = (rids // cfg.nsh) * (2 * cfg.nshp) + cfg.nshp + rids % cfg.nsh
        cids = (cids // cfg.nsh) * (2 * cfg.nshp) + cids % cfg.nsh
        eidx = np.concatenate(
            [_wrap_idx(rids, cfg), _wrap_idx(cids, cfg)], axis=1
        )
        per_core.append({"zin": zsh, "win": wsh, "eidx": eidx})
    return per_core


_NC_CACHE = {}


def get_nc(cfg: Cfg):
    key = (cfg.gchunk,)
    if key not in _NC_CACHE:
        _NC_CACHE[key] = build_kernel(cfg)
    return _NC_CACHE[key]


def _unshard(results, cfg: Cfg) -> np.ndarray:
    parts = []
    for c in range(cfg.n_cores):
        raw = results[c]["scores"]  # [128, eblocks], edge i at [i%128, i//128]
        parts.append(raw.T.reshape(-1)[: cfg.e_core])
    return np.concatenate(parts).astype(np.float32)


def run(z_drug, weight, batch_edges, cfg: Cfg, repeats: int = 1):
    """Returns (scores[200000] f32, [wall seconds per call])."""
    import time

    from concourse.bass_utils import run_bass_kernel_spmd

    nc = get_nc(cfg)
    in_maps = prep_inputs(z_drug, weight, batch_edges, cfg)
    walls = []
    res = None
    for _ in range(max(1, repeats)):
        t0 = time.perf_counter()
        try:
            res = run_bass_kernel_spmd(
                nc, in_maps, core_ids=list(range(cfg.n_cores))
            )
        except Exception:
            if res is not None:
                break  # keep earlier good result; a repeat run hiccupped
            time.sleep(30)
            res = run_bass_kernel_spmd(
                nc, in_maps, core_ids=list(range(cfg.n_cores))
            )
        walls.append(time.perf_counter() - t0)
    return _unshard(res.results, cfg), walls


def kernel(z_drug, weight, batch_edges):
    out, _ = run(z_drug, weight, batch_edges, CFG)
    return out


# revision 16
# speedup vs baseline: 1.8659x; 1.8659x over previous
"""Bilinear decoder kernel for Trainium2 (8 NeuronCores).

score_e = sigmoid(z[row_e] @ W @ z[col_e])  for 200k edges, d=512.

v2 strategy (host->device transfer over axon is ~40MB/s, so uploads are
sharded and the tables are rebuilt on-device with AllGathers):
  - Upload per core: z shard [1280,512] bf16 (1/8 of nodes), W shard
    [64,512] bf16 (1/8 of rows), edge indices [16, 2*1568] int16.
    ~1.6MB/core vs ~41MB/core for the replicated-f32 baseline.
  - Device: AllGather W (tiny) -> full W in SBUF. Load z^T via
    dma_start_transpose, matmul ZW_c = z_c @ W for the local 1280-node
    shard (tensor engine, bf16). AllGather z -> full Z table [10240,512]
    in DRAM; AllGather ZW_c -> full ZW table.
  - Gather ZW[row_e] and Z[col_e] rows via dma_gather, per-edge dot via
    DVE mul + ACT copy-with-accumulate, sigmoid on ACT.
  - Edges sharded 25000/core; node ids are remapped on host to the
    padded AllGather layout (node n -> (n//1250)*1280 + n%1250).

Host-side work is layout-only: bf16 casts, shard slicing, index
wrap/remap, output unshard.
"""

import sys

if "/opt/trn_rl_repo" not in sys.path:
    sys.path.insert(0, "/opt/trn_rl_repo")

from dataclasses import dataclass

import numpy as np


@dataclass(frozen=True)
class Cfg:
    n_cores: int = 8
    d: int = 512              # embedding dim
    n_nodes: int = 10000      # node table rows
    e_total: int = 200000     # total edges
    gchunk: int = 512         # edges per dma_gather (SDMA packet limit:
    #                           512 rows = 32 descriptors/engine works,
    #                           1024+ faults the exec unit)

    @property
    def kb(self):
        return self.d // 128  # 4

    @property
    def nsh(self):
        return self.n_nodes // self.n_cores  # 1250 nodes per core

    @property
    def nshp(self):
        return ((self.nsh + 127) // 128) * 128  # 1280 padded

    @property
    def nblocks(self):
        return self.nshp // 128  # 10

    @property
    def ntab(self):
        return self.nshp * self.n_cores  # 10240 table rows

    @property
    def wsh(self):
        return self.d // self.n_cores  # 64 W rows per core

    @property
    def e_core(self):
        return self.e_total // self.n_cores  # 25000

    @property
    def ep_core(self):
        return ((self.e_core + 127) // 128) * 128  # 25088

    @property
    def eblocks(self):
        return self.ep_core // 128  # 196

    @property
    def idx_cols(self):
        return self.ep_core // 16  # 1568

    @property
    def chunks(self):
        out = []
        left = self.ep_core
        while left > 0:
            c = min(self.gchunk, left)
            out.append(c)
            left -= c
        return out


CFG = Cfg()


def build_kernel(cfg: Cfg):
    """Build + compile the Bacc module. Returns nc."""
    import concourse.bacc as bacc
    import concourse.mybir as mybir
    from concourse import tile

    f32 = mybir.dt.float32
    f16 = mybir.dt.float16
    bf16 = mybir.dt.bfloat16
    i16 = mybir.dt.int16

    D, KB, NSHP, NB = cfg.d, cfg.kb, cfg.nshp, cfg.nblocks
    NTAB, WSH, IC = cfg.ntab, cfg.wsh, cfg.idx_cols
    group = [list(range(cfg.n_cores))]

    nc = bacc.Bacc(
        "TRN2", target_bir_lowering=False, debug=False, num_devices=cfg.n_cores
    )

    zin = nc.dram_tensor("zin", [NSHP, D], bf16, kind="ExternalInput")
    win = nc.dram_tensor("win", [WSH, D], bf16, kind="ExternalInput")
    eidx = nc.dram_tensor("eidx", [16, 2 * IC], i16, kind="ExternalInput")
    scores = nc.dram_tensor("scores", [128, cfg.eblocks], f16, kind="ExternalOutput")
    # Shared-scratchpad AllGather outputs: one copy in chip HBM instead of
    # eight core-local replicas (supported for AllGather with 8 cores).
    wag_out = nc.dram_tensor("wag_out", [D, D], bf16, addr_space="Shared")
    zag_out = nc.dram_tensor("zag_out", [2 * NTAB, D], bf16, addr_space="Shared")

    with tile.TileContext(nc) as tc:
        with (
            tc.tile_pool(name="const", bufs=1) as constp,
            tc.tile_pool(name="dram", bufs=1, space="DRAM") as dramp,
            tc.tile_pool(name="zwsb", bufs=2) as zwp,
            tc.tile_pool(name="rows", bufs=3) as rowsp,
            tc.tile_pool(name="cols", bufs=3) as colsp,
            tc.tile_pool(name="prod", bufs=4) as prodp,
            tc.tile_pool(name="ps", bufs=2, space="PSUM") as psp,
        ):
            # ---- DRAM bounce buffers (collectives can't touch I/O tensors) ----
            # Combined z+zw AllGather: each core contributes [z_c; zw_c]
            # [2*NSHP, D]; output is the interleaved table
            # [z_0; zw_0; z_1; zw_1; ...] that host-side index remap targets.
            wag_in = dramp.tile([WSH, D], bf16, tag="wag_in")
            zag_in = dramp.tile([2 * NSHP, D], bf16, tag="zag_in")

            nc.sync.dma_start(wag_in[:], win.ap())
            nc.sync.dma_start(zag_in[:NSHP, :], zin.ap())

            # ---- collectives (gpsimd, straight-line order) ----
            nc.gpsimd.collective_compute(
                "AllGather",
                mybir.AluOpType.bypass,
                replica_groups=group,
                ins=[wag_in.opt()],
                outs=[wag_out.ap()],
            )

            # ---- SBUF constants ----
            w_sb = constp.tile([128, KB, D], bf16, tag="w")
            nc.sync.dma_start(
                w_sb[:], wag_out.ap().rearrange("(kb p) f -> p kb f", p=128)
            )
            # z^T for the matmul's stationary operand: [128, kb, NSHP]
            zt_sb = constp.tile([128, KB, NSHP], bf16, tag="zt")
            nc.sync.dma_start_transpose(zt_sb[:], zin.ap())
            # edge indices: upload 16-partition wrap, replicate to 128
            idx_sb = constp.tile([128, 2 * IC], i16, tag="idx")
            nc.sync.dma_start(idx_sb[0:16, :], eidx.ap())
            for r in range(1, 8):
                nc.sync.dma_start(idx_sb[16 * r : 16 * (r + 1), :], idx_sb[0:16, :])
            scores_sb = constp.tile([128, cfg.eblocks], f32, tag="scores")
            sig_sb = constp.tile([128, cfg.eblocks], f16, tag="sig")
            scratch = constp.tile([128, D], f32, tag="scratch")

            # ---- phase 1: ZW_c = z_c @ W for the local node shard ----
            for nb in range(NB):
                ps = psp.tile([128, D], f32, tag="ps")
                for kb in range(KB):
                    nc.tensor.matmul(
                        ps[:],
                        lhsT=zt_sb[:, kb, nb * 128 : (nb + 1) * 128],
                        rhs=w_sb[:, kb, :],
                        start=(kb == 0),
                        stop=(kb == KB - 1),
                    )
                zw_t = zwp.tile([128, D], bf16, tag="zwt")
                nc.vector.tensor_copy(zw_t[:], ps[:])
                nc.sync.dma_start(
                    zag_in[NSHP + nb * 128 : NSHP + (nb + 1) * 128, :], zw_t[:]
                )

            nc.gpsimd.collective_compute(
                "AllGather",
                mybir.AluOpType.bypass,
                replica_groups=group,
                ins=[zag_in.opt()],
                outs=[zag_out.ap()],
            )

            # ---- phase 2: gathers + per-edge dots ----
            blk = 0
            off = 0
            for G in cfg.chunks:
                gb = G // 128
                ctile = colsp.tile([128, cfg.gchunk // 128, D], bf16, tag="ct")
                nc.gpsimd.dma_gather(
                    ctile[:, :gb, :],
                    zag_out.ap(),
                    idx_sb[:, IC + off : IC + off + G // 16],
                    num_idxs=G,
                    num_idxs_reg=G,
                    elem_size=D,
                )
                rtile = rowsp.tile([128, cfg.gchunk // 128, D], bf16, tag="rt")
                nc.gpsimd.dma_gather(
                    rtile[:, :gb, :],
                    zag_out.ap(),
                    idx_sb[:, off : off + G // 16],
                    num_idxs=G,
                    num_idxs_reg=G,
                    elem_size=D,
                )
                for b in range(gb):
                    # DVE multiply, then ACT copy-with-accumulate = free-dim sum.
                    # bf16 products: 2x DVE/ACT rate; ACT still accumulates f32.
                    prod = prodp.tile([128, D], bf16, tag="prod")
                    nc.vector.tensor_mul(prod[:], rtile[:, b, :], ctile[:, b, :])
                    nc.scalar.activation(
                        scratch[:],
                        prod[:],
                        mybir.ActivationFunctionType.Copy,
                        accum_out=scores_sb[:, blk : blk + 1],
                    )
                    blk += 1
                off += G // 16

            # ---- sigmoid + writeback ----
            nc.scalar.activation(
                sig_sb[:], scores_sb[:], mybir.ActivationFunctionType.Sigmoid
            )
            nc.sync.dma_start(scores.ap(), sig_sb[:])

    nc.compile()
    return nc


def _wrap_idx(ids: np.ndarray, cfg: Cfg) -> np.ndarray:
    """int table-row ids [ep_core] -> [16, ep_core//16] int16 in the
    16-partition wrapped layout dma_gather expects."""
    out = np.empty((16, cfg.ep_core // 16), dtype=np.int16)
    off = 0
    for G in cfg.chunks:
        c = ids[off : off + G].reshape(G // 16, 16).T  # [16, G/16]
        out[:, off // 16 : (off + G) // 16] = c
        off += G
    return out


def prep_inputs(z_drug, weight, batch_edges, cfg: Cfg):
    """Host-side layout prep. Returns per-core input maps."""
    import ml_dtypes

    bf16 = ml_dtypes.bfloat16

    z = np.asarray(z_drug, dtype=np.float32)
    w = np.asarray(weight, dtype=np.float32)
    be = np.asarray(batch_edges)

    per_core = []
    for c in range(cfg.n_cores):
        # z shard: nodes [c*1250, (c+1)*1250), padded to 1280 rows
        zsh = np.zeros((cfg.nshp, cfg.d), dtype=bf16)
        zsh[: cfg.nsh] = z[c * cfg.nsh : (c + 1) * cfg.nsh].astype(bf16)
        # W shard: rows [c*64, (c+1)*64)
        wsh = np.ascontiguousarray(
            w[c * cfg.wsh : (c + 1) * cfg.wsh].astype(bf16)
        )
        # edge shard + remap node ids to the padded AllGather table layout
        sl = slice(c * cfg.e_core, (c + 1) * cfg.e_core)
        rids = np.zeros(cfg.ep_core, dtype=np.int64)
        cids = np.zeros(cfg.ep_core, dtype=np.int64)
        rids[: cfg.e_core] = be[0, sl]
        cids[: cfg.e_core] = be[1, sl]
        # combined table layout: [z_0; zw_0; z_1; zw_1; ...], stride 2*nshp
        rids = (rids // cfg.nsh) * (2 * cfg.nshp) + cfg.nshp + rids % cfg.nsh
        cids = (cids // cfg.nsh) * (2 * cfg.nshp) + cids % cfg.nsh
        eidx = np.concatenate(
            [_wrap_idx(rids, cfg), _wrap_idx(cids, cfg)], axis=1
        )
        per_core.append({"zin": zsh, "win": wsh, "eidx": eidx})
    return per_core


_NC_CACHE = {}


def get_nc(cfg: Cfg):
    key = (cfg.gchunk,)
    if key not in _NC_CACHE:
        _NC_CACHE[key] = build_kernel(cfg)
    return _NC_CACHE[key]


def _unshard(results, cfg: Cfg) -> np.ndarray:
    parts = []
    for c in range(cfg.n_cores):
        raw = results[c]["scores"]  # [128, eblocks], edge i at [i%128, i//128]
        parts.append(raw.T.reshape(-1)[: cfg.e_core])
    return np.concatenate(parts).astype(np.float32)


def run(z_drug, weight, batch_edges, cfg: Cfg, repeats: int = 1):
    """Returns (scores[200000] f32, [wall seconds per call])."""
    import time

    from concourse.bass_utils import run_bass_kernel_spmd

    nc = get_nc(cfg)
    in_maps = prep_inputs(z_drug, weight, batch_edges, cfg)
    walls = []
    res = None
    for _ in range(max(1, repeats)):
        t0 = time.perf_counter()
        try:
            res = run_bass_kernel_spmd(
                nc, in_maps, core_ids=list(range(cfg.n_cores))
            )
        except Exception:
            if res is not None:
                break  # keep earlier good result; a repeat run hiccupped
            time.sleep(30)
            res = run_bass_kernel_spmd(
                nc, in_maps, core_ids=list(range(cfg.n_cores))
            )
        walls.append(time.perf_counter() - t0)
    return _unshard(res.results, cfg), walls


def kernel(z_drug, weight, batch_edges):
    out, _ = run(z_drug, weight, batch_edges, CFG)
    return out
